# revision 1
# baseline (speedup 1.0000x reference)
"""Trainium2 Bass kernel for nn_ConvAttention: LayerNorm -> 1x1-conv QKV ->
per-(b,h)-row attention over W -> skip connection.

Sharding: data-parallel over batch B=8 across 8 NeuronCores. Each core
processes 64 (h) slabs of [W=256, C=256].

Numerics strategy (fp32 reference): all matmuls run in bf16 at full PE rate
(FWL weight loads), with two-term hi/lo operand splits and 3-term products
reconstructing ~fp32 precision (error ~2^-18 per product). This matters
because scores ~ N(0, 128) feed exp(), which amplifies absolute score error
into relative softmax-weight error. The softmax weights themselves are kept
at f32r precision (~2^-12), which the normalized output tolerates.

Softmax max-subtraction is replaced by a constant shift (exact in real
arithmetic; scores are bounded well inside fp32 exp range), which lets us
compute only TRANSPOSED scores s^T = k @ q^T and exponentiate those directly
-- no PE transpose of the softmax weights. Z comes for free from a ones
column appended to the V operand of the output matmul.
"""

import os
import sys

for _p in ("/opt/trn_rl_repo", "/root/.axon_site/_ro/trn_rl_repo"):
    if _p not in sys.path:
        sys.path.insert(0, _p)

import numpy as np

import concourse.tile as tile
from concourse import bacc, mybir
from concourse.bass_utils import run_bass_kernel_spmd
from concourse.masks import make_identity

F32 = mybir.dt.float32
F32R = mybir.dt.float32r
BF16 = mybir.dt.bfloat16
AF = mybir.ActivationFunctionType
ALU = mybir.AluOpType

B, H, W, C = 8, 64, 256, 256
F2 = 2 * C
NS = H  # slabs per core (batch-sharded over 8 cores)
EPS = 1e-3  # Keras LayerNormalization default
SHIFT = 32.0  # constant softmax shift (replaces per-row max subtraction)

_NC_CACHE: dict = {}


def _install_act_root():
    """Reorder act_info.json so natural_log_exp_and_others is the first set:
    bass' first-match table chooser then resolves both Ln and Exp to that one
    set instead of alternating exp_and_others / natural_log every slab
    (129 table loads x ~2.7us). Both bass (get_activation_tables) and walrus
    (--act-root-json via BASS_ACT_ROOT_JSON_PATH) must read the same file so
    the pre-placed set ids stay in range."""
    if os.environ.get("BASS_ACT_ROOT_JSON_PATH"):
        return
    try:
        import json
        import tempfile

        import neuronxcc.driver.jobs.support.FindActInfo as FAI
        from neuronxcc.driver.Job import Job

        src = FAI.findActInfoFile(Job.getPackageDir(), "gen3")
        srcdir = os.path.dirname(src)
        d = json.load(open(src))
        sets = d["act_func_sets"]
        first = [s for s in sets if s["name"] == "natural_log_exp_and_others"]
        if not first:
            return
        rest = [s for s in sets if s["name"] != "natural_log_exp_and_others"]
        d["act_func_sets"] = first + rest
        td = tempfile.mkdtemp(prefix="act_root_")
        for fn in os.listdir(srcdir):
            sp = os.path.join(srcdir, fn)
            if os.path.isfile(sp) and fn != os.path.basename(src):
                os.symlink(sp, os.path.join(td, fn))
        out = os.path.join(td, os.path.basename(src))
        with open(out, "w") as f:
            json.dump(d, f)
        os.environ["BASS_ACT_ROOT_JSON_PATH"] = out
        # bass side reads via findActInfoFile; point it at the same file
        _orig = FAI.findActInfoFile
        FAI.findActInfoFile = lambda *a, **k: out
        import concourse.hw_specs as hw_specs

        hw_specs.get_activation_tables.cache_clear()
    except Exception as e:  # noqa: BLE001
        print(f"act root override failed (table thrash will persist): {e}")


def _build(with_bias: bool):
    _install_act_root()
    nc = bacc.Bacc("TRN2", target_bir_lowering=False, debug=False, num_devices=8)
    x_d = nc.dram_tensor("x", [NS, W, C], F32, kind="ExternalInput").ap()
    wqk_h_d = nc.dram_tensor("wqk_h", [2, 128, 256], BF16, kind="ExternalInput").ap()
    wqk_l_d = nc.dram_tensor("wqk_l", [2, 128, 256], BF16, kind="ExternalInput").ap()
    wv_h_d = nc.dram_tensor("wv_h", [2, 128, 256], BF16, kind="ExternalInput").ap()
    wv_l_d = nc.dram_tensor("wv_l", [2, 128, 256], BF16, kind="ExternalInput").ap()
    bqk_d = bv_d = None
    if with_bias:
        bqk_d = nc.dram_tensor("bqk", [2, 128], F32, kind="ExternalInput").ap()
        bv_d = nc.dram_tensor("bv", [256], F32, kind="ExternalInput").ap()
    out_d = nc.dram_tensor("out", [NS, W, C], F32, kind="ExternalOutput").ap()

    # per-slab views: [p=128, t(w-chunk)=2, c=256]
    x_r = x_d.rearrange("s (t p) c -> s p t c", p=128)
    out_r = out_d.rearrange("s (t p) c -> s p t c", p=128)

    with tile.TileContext(nc) as tc:
        _emit(nc, tc, x_r, out_r,
              (wqk_h_d, wqk_l_d, wv_h_d, wv_l_d), bqk_d, bv_d)
    nc.compile()
    return nc


def _emit(nc, tc, x_r, out_r, w_ds, bqk_d, bv_d):
    from contextlib import ExitStack

    wqk_h_d, wqk_l_d, wv_h_d, wv_l_d = w_ds
    with ExitStack() as ctx:
        ec = ctx.enter_context
        consts = ec(tc.tile_pool(name="consts", bufs=1))
        xpool = ec(tc.tile_pool(name="xp", bufs=5))
        xnpool = ec(tc.tile_pool(name="xnp", bufs=3))
        xtpool = ec(tc.tile_pool(name="xtp", bufs=3))
        qkpool = ec(tc.tile_pool(name="qkp", bufs=3))
        epool = ec(tc.tile_pool(name="ep", bufs=3))
        vpool = ec(tc.tile_pool(name="vp", bufs=3))
        opool = ec(tc.tile_pool(name="op", bufs=4))
        stat = ec(tc.tile_pool(name="stat", bufs=6))
        ps_xnT = ec(tc.tile_pool(name="ps_xnT", bufs=2, space="PSUM"))
        ps_qk = ec(tc.tile_pool(name="ps_qk", bufs=1, space="PSUM"))
        ps_sT = ec(tc.tile_pool(name="ps_sT", bufs=1, space="PSUM"))
        ps_v = ec(tc.tile_pool(name="ps_v", bufs=2, space="PSUM"))
        ps_y = ec(tc.tile_pool(name="ps_y", bufs=1, space="PSUM"))

        ident = consts.tile([128, 128], F32)
        make_identity(nc, ident)
        negshift = consts.tile([128, 1], F32)
        nc.vector.memset(negshift, -SHIFT)
        eps_t = consts.tile([128, 1], F32)
        nc.vector.memset(eps_t, EPS)

        wqk_h = consts.tile([128, 2, 256], BF16)
        nc.sync.dma_start(wqk_h, wqk_h_d.rearrange("t p f -> p t f"))
        wqk_l = consts.tile([128, 2, 256], BF16)
        nc.sync.dma_start(wqk_l, wqk_l_d.rearrange("t p f -> p t f"))
        wv_h = consts.tile([128, 2, 256], BF16)
        nc.sync.dma_start(wv_h, wv_h_d.rearrange("t p f -> p t f"))
        wv_l = consts.tile([128, 2, 256], BF16)
        nc.sync.dma_start(wv_l, wv_l_d.rearrange("t p f -> p t f"))

        if bqk_d is not None:
            bqk_sb = consts.tile([128, 2], F32)
            nc.sync.dma_start(bqk_sb, bqk_d.rearrange("t p -> p t"))
            import concourse.bass as bass
            bvf = consts.tile([128, 2, 256], F32)
            bv_b = bass.AP(tensor=bv_d.tensor, offset=bv_d.offset,
                           ap=[[0, 128], [0, 2], [1, 256]])
            nc.sync.dma_start(bvf, bv_b)

        def emit_tail(pv):
            """Slab tail, software-pipelined one slab behind: y-matmuls,
            1/Z normalize, skip-add, store. Emitting it one iteration later
            keeps PE from stalling on the exp -> Eh/El extraction chain."""
            Eh, El, vh, vl, x_sb, s = (
                pv["Eh"], pv["El"], pv["vh"], pv["vl"], pv["x_sb"], pv["s"])
            # y = E^T.T @ [v | 1]  (3-term; col 256/257 accumulate Z)
            p_y = ps_y.tile([128, 2, 512], F32)
            for it in (0, 1):
                i = 0
                for jt in (0, 1):
                    trip = ((Eh, vh, 258), (Eh, vl, 256), (El, vh, 258))
                    for (le, rv, n) in trip:
                        nc.tensor.matmul(
                            p_y[:, it, 0:n],
                            le[:, jt, it * 128:(it + 1) * 128],
                            rv[:, jt, 0:n],
                            start=(i == 0), stop=(i == 5),
                            skip_group_check=True)
                        i += 1
            rZ = stat.tile([128, 2, 1], F32)
            nc.vector.reciprocal(rZ, p_y[:, :, 256:257])
            # out = x + y * rZ
            tmp = opool.tile([128, 2, 256], F32)
            for it in (0, 1):
                nc.scalar.mul(tmp[:, it, :], p_y[:, it, 0:256], rZ[:, it, :])
            o_sb = opool.tile([128, 2, 256], F32)
            nc.gpsimd.tensor_tensor(out=o_sb, in0=tmp, in1=x_sb, op=ALU.add)
            nc.sync.dma_start(out_r[s], o_sb)

        def emit_scores(pq):
            """Scores stage, software-pipelined one slab behind: s^T matmuls,
            exp, softmax-weight hi/lo extraction. Deferring it means the PE
            never waits on this slab's qh/ql extraction chain."""
            qh, ql = pq["qh"], pq["ql"]
            # s^T = k @ q^T  (3-term), then E^T = exp(s^T - SHIFT) in f32r
            p_sT = ps_sT.tile([128, 2, 256], F32)
            for jt in (0, 1):
                terms = ((qh, qh), (qh, ql), (ql, qh))
                for i, (kt, qt) in enumerate(terms):
                    nc.tensor.matmul(
                        p_sT[:, jt, :],
                        kt[:, 1, jt * 128:(jt + 1) * 128],
                        qt[:, 0, :],
                        start=(i == 0), stop=(i == len(terms) - 1))
            E = epool.tile([128, 2, 256], F32R)
            nc.scalar.activation(out=E, in_=p_sT, func=AF.Exp,
                                 bias=negshift, scale=1.0)
            Eh = epool.tile([128, 2, 256], BF16)
            nc.vector.tensor_copy(Eh, E)
            El = epool.tile([128, 2, 256], BF16)
            nc.gpsimd.tensor_tensor(out=El, in0=E, in1=Eh, op=ALU.subtract)
            return {"Eh": Eh, "El": El, "vh": pq["vh"], "vl": pq["vl"],
                    "x_sb": pq["x_sb"], "s": pq["s"]}

        prev = None
        prevq = None
        for s in range(NS):
            x_sb = xpool.tile([128, 2, 256], F32)
            nc.sync.dma_start(x_sb, x_r[s])

            # LayerNorm stats per row (partition = w position)
            st = stat.tile([128, 2, 6], F32)
            mv = stat.tile([128, 2, 2], F32)
            for t in (0, 1):
                nc.vector.bn_stats(st[:, t, :], x_sb[:, t, :])
                nc.vector.bn_aggr(mv[:, t, :], st[:, t, :])
            # rs = rsqrt(var + eps) = exp(-0.5 * ln(var + eps)); ln+exp live
            # in one ACT table set (see _install_act_root)
            lnv = stat.tile([128, 2, 1], F32)
            nc.scalar.activation(out=lnv, in_=mv[:, :, 1:2], func=AF.Ln,
                                 bias=eps_t, scale=1.0)
            rs = stat.tile([128, 2, 1], F32)
            nc.scalar.activation(out=rs, in_=lnv, func=AF.Exp, scale=-0.5)

            # xn = (x - mean) * rs   (gamma/beta folded into weights on host)
            # two single-scalar ops: dual-scalar tensor_scalar runs 1x mode
            # (~792ns) while single-scalar hits 2x_2P (~194ns each)
            xn = xnpool.tile([128, 2, 256], F32)
            for t in (0, 1):
                nc.vector.tensor_scalar_sub(xn[:, t, :], x_sb[:, t, :],
                                            mv[:, t, 0:1])
            for t in (0, 1):
                nc.vector.tensor_scalar_mul(xn[:, t, :], xn[:, t, :],
                                            rs[:, t, :])

            # transpose xn -> [c, w] layout (PE transpose, fp32-exact),
            # then bf16 hi/lo split
            p_xnT = ps_xnT.tile([128, 2, 256], F32)
            for cc in (0, 1):
                for t in (0, 1):
                    nc.tensor.transpose(
                        p_xnT[:, cc, t * 128:(t + 1) * 128],
                        xn[:, t, cc * 128:(cc + 1) * 128], ident)
            xh = xtpool.tile([128, 2, 256], BF16)
            nc.scalar.copy(xh, p_xnT)
            xl = xtpool.tile([128, 2, 256], BF16)
            nc.vector.tensor_tensor(out=xl, in0=p_xnT, in1=xh, op=ALU.subtract)

            # previous slab's tail fills PE while this slab's splits extract
            if prev is not None:
                emit_tail(prev)

            # qk^T = Wqk^T @ xn^T   (3-term bf16 split: ~fp32 precision)
            p_qk = ps_qk.tile([128, 2, 256], F32)
            for blk in (0, 1):
                # xh-only terms first: xl (DVE residual extraction) lands
                # later than xh (ACT copy), so xl-dependent matmuls go last
                terms = [(wqk_h, xh, cc) for cc in (0, 1)] +                         [(wqk_l, xh, cc) for cc in (0, 1)] +                         [(wqk_h, xl, cc) for cc in (0, 1)]
                for i, (lw, rx, cc) in enumerate(terms):
                    nc.tensor.matmul(
                        p_qk[:, blk, :],
                        lw[:, cc, blk * 128:(blk + 1) * 128],
                        rx[:, cc, :],
                        start=(i == 0), stop=(i == len(terms) - 1))
            if bqk_d is not None:
                for blk in (0, 1):
                    nc.vector.tensor_scalar(
                        out=p_qk[:, blk, :], in0=p_qk[:, blk, :],
                        scalar1=bqk_sb[:, blk:blk + 1], scalar2=None,
                        op0=ALU.add)
            qh = qkpool.tile([128, 2, 256], BF16)
            nc.scalar.copy(qh, p_qk)
            ql = qkpool.tile([128, 2, 256], BF16)
            nc.vector.tensor_tensor(out=ql, in0=p_qk, in1=qh, op=ALU.subtract)

            # v = xn @ Wv  (3-term bf16), with a ones column appended to the
            # hi part so the y-matmul also produces Z = sum_j E[j, i]
            p_v = ps_v.tile([128, 2, 256], F32)
            for jt in (0, 1):
                terms = [(xh, wv_h, cc) for cc in (0, 1)] +                         [(xh, wv_l, cc) for cc in (0, 1)] +                         [(xl, wv_h, cc) for cc in (0, 1)]
                for i, (lx, rw, cc) in enumerate(terms):
                    nc.tensor.matmul(
                        p_v[:, jt, :],
                        lx[:, cc, jt * 128:(jt + 1) * 128],
                        rw[:, cc, :],
                        start=(i == 0), stop=(i == len(terms) - 1))
            vh = vpool.tile([128, 2, 258], BF16)
            if bv_d is not None:
                nc.scalar.activation(out=vh[:, :, 0:256], in_=p_v,
                                     func=AF.Identity, bias=0.0, scale=1.0)
                # bias must be added before rounding; cheap fallback path:
                nc.vector.tensor_tensor(out=vh[:, :, 0:256], in0=p_v, in1=bvf,
                                        op=ALU.add)
                vl = vpool.tile([128, 2, 258], BF16)
                # vl = (v + bias) - vh; compute v+bias into scratch first
                vb = vpool.tile([128, 2, 256], F32)
                nc.vector.tensor_tensor(out=vb, in0=p_v, in1=bvf, op=ALU.add)
                nc.vector.tensor_tensor(out=vl[:, :, 0:256], in0=vb, in1=vh[:, :, 0:256],
                                        op=ALU.subtract)
            else:
                nc.scalar.copy(vh[:, :, 0:256], p_v)
                vl = vpool.tile([128, 2, 258], BF16)
                nc.vector.tensor_tensor(out=vl[:, :, 0:256], in0=p_v,
                                        in1=vh[:, :, 0:256], op=ALU.subtract)
            nc.gpsimd.memset(vh[:, :, 256:258], 1.0)

            if prevq is not None:
                prev = emit_scores(prevq)

            prevq = {"qh": qh, "ql": ql, "vh": vh, "vl": vl,
                     "x_sb": x_sb, "s": s}
        emit_tail(prev)
        prev = emit_scores(prevq)
        emit_tail(prev)


def _install_ntff_hook():
    """Register the axon NTFF profiling hook (the image's antenv lacks
    axon_hooks, so boot skipped registration). Trace-only; best-effort."""
    try:
        import types

        import antenv

        if getattr(antenv, "axon_hooks", None) is not None:
            return
        mod = types.ModuleType("antenv.axon_hooks")
        _h = [None]
        mod.set_axon_ntff_profile_hook = lambda h: _h.__setitem__(0, h)
        mod.get_axon_ntff_profile_hook = lambda: _h[0]
        sys.modules["antenv.axon_hooks"] = mod
        antenv.axon_hooks = mod
        from trn_agent_boot.trn_boot import _ntff_profile_via_ctypes

        hook = _ntff_profile_via_ctypes("/opt/axon/libaxon_pjrt.so")
        if hook is not None:
            mod.set_axon_ntff_profile_hook(hook)
    except Exception as e:  # noqa: BLE001
        print(f"ntff hook install failed (timing unavailable): {e}")


def _bf16_split(a):
    import ml_dtypes

    hi = a.astype(ml_dtypes.bfloat16)
    lo = (a - hi.astype(np.float64)).astype(ml_dtypes.bfloat16)
    return hi, lo


def kernel(x, ln_gamma, ln_beta, W_qkv):
    x = np.asarray(x, dtype=np.float32)
    ln_gamma = np.asarray(ln_gamma, dtype=np.float32)
    ln_beta = np.asarray(ln_beta, dtype=np.float32)
    W_qkv = np.asarray(W_qkv, dtype=np.float32)
    assert x.shape == (B, H, W, C) and W_qkv.shape == (C, F2)

    # fold gamma/beta into the projection (1x1 conv has no bias of its own)
    Wp = (ln_gamma.astype(np.float64)[:, None] * W_qkv.astype(np.float64))
    bW = (ln_beta.astype(np.float64) @ W_qkv.astype(np.float64)).astype(np.float32)
    with_bias = bool(np.any(bW != 0.0))

    key = with_bias
    if key not in _NC_CACHE:
        _NC_CACHE[key] = _build(with_bias)
    nc = _NC_CACHE[key]

    wqk_h, wqk_l = _bf16_split(Wp[:, :256])
    wv_h, wv_l = _bf16_split(Wp[:, 256:])
    in_maps = []
    for b in range(B):
        m = {
            "x": np.ascontiguousarray(x[b]),
            "wqk_h": np.ascontiguousarray(wqk_h.reshape(2, 128, 256)),
            "wqk_l": np.ascontiguousarray(wqk_l.reshape(2, 128, 256)),
            "wv_h": np.ascontiguousarray(wv_h.reshape(2, 128, 256)),
            "wv_l": np.ascontiguousarray(wv_l.reshape(2, 128, 256)),
        }
        if with_bias:
            m["bqk"] = np.ascontiguousarray(bW[:256].reshape(2, 128))
            m["bv"] = np.ascontiguousarray(bW[256:])
        in_maps.append(m)

    trace = os.environ.get("KERNEL_TRACE", "") == "1"
    if trace:
        _install_ntff_hook()
    res = run_bass_kernel_spmd(nc, in_maps, core_ids=list(range(B)), trace=trace)
    if trace and res.exec_time_ns is not None:
        print(f"HW exec time: {res.exec_time_ns} ns")
        if res.instructions_and_trace is not None:
            print(f"trace: {res.instructions_and_trace[1]}")
    out = np.stack([res.results[b]["out"] for b in range(B)], axis=0)
    return out.reshape(B, H, W, C).astype(np.float32, copy=False)



# revision 2
# speedup vs baseline: 7.4171x; 7.4171x over previous
"""Trainium2 Bass kernel for nn_ConvAttention: LayerNorm -> 1x1-conv QKV ->
per-(b,h)-row attention over W -> skip connection.

Sharding: data-parallel over batch B=8 across 8 NeuronCores. Each core
processes 64 (h) slabs of [W=256, C=256].

Numerics strategy: all matmuls run in float32r (TF32-like rounded fp32),
which the PE streams at 1 cycle/row when the moving free dim is >= 256 --
full bf16 rate at ~fp32-ish precision. This removes the 3-term bf16 hi/lo
split matmuls AND all the hi/lo extraction traffic on ACT/DVE that the
previous version needed. Softmax max-subtraction is replaced by a constant
shift (exact in real arithmetic); scores are computed transposed
(s^T = k @ q^T) so the exp output feeds the y-matmul directly as lhsT.
Z comes from a ones column appended to the V operand.
"""

import os
import sys

for _p in ("/opt/trn_rl_repo", "/root/.axon_site/_ro/trn_rl_repo"):
    if _p not in sys.path:
        sys.path.insert(0, _p)

import numpy as np

import concourse.tile as tile
from concourse import bacc, mybir
from concourse.bass_utils import run_bass_kernel_spmd
from concourse.masks import make_identity

F32 = mybir.dt.float32
F32R = mybir.dt.float32r
BF16 = mybir.dt.bfloat16
AF = mybir.ActivationFunctionType
ALU = mybir.AluOpType

B, H, W, C = 8, 64, 256, 256
F2 = 2 * C
NS = H  # slabs per core (batch-sharded over 8 cores)
EPS = 1e-3  # Keras LayerNormalization default
SHIFT = 32.0  # constant softmax shift (replaces per-row max subtraction)

_NC_CACHE: dict = {}


def _install_act_root():
    """Reorder act_info.json so natural_log_exp_and_others is the first set:
    bass' first-match table chooser then resolves Ln, Exp, Identity and Copy
    to one set, avoiding per-slab ACT table reloads (~2.7us each)."""
    if os.environ.get("BASS_ACT_ROOT_JSON_PATH"):
        return
    try:
        import json
        import tempfile

        import neuronxcc.driver.jobs.support.FindActInfo as FAI
        from neuronxcc.driver.Job import Job

        src = FAI.findActInfoFile(Job.getPackageDir(), "gen3")
        srcdir = os.path.dirname(src)
        d = json.load(open(src))
        sets = d["act_func_sets"]
        first = [s for s in sets if s["name"] == "natural_log_exp_and_others"]
        if not first:
            return
        rest = [s for s in sets if s["name"] != "natural_log_exp_and_others"]
        d["act_func_sets"] = first + rest
        td = tempfile.mkdtemp(prefix="act_root_")
        for fn in os.listdir(srcdir):
            sp = os.path.join(srcdir, fn)
            if os.path.isfile(sp) and fn != os.path.basename(src):
                os.symlink(sp, os.path.join(td, fn))
        out = os.path.join(td, os.path.basename(src))
        with open(out, "w") as f:
            json.dump(d, f)
        os.environ["BASS_ACT_ROOT_JSON_PATH"] = out
        _orig = FAI.findActInfoFile
        FAI.findActInfoFile = lambda *a, **k: out
        import concourse.hw_specs as hw_specs

        hw_specs.get_activation_tables.cache_clear()
    except Exception as e:  # noqa: BLE001
        print(f"act root override failed (table thrash will persist): {e}")


def _build(with_bias: bool):
    _install_act_root()
    nc = bacc.Bacc("TRN2", target_bir_lowering=False, debug=False, num_devices=8)
    x_d = nc.dram_tensor("x", [NS, W, C], F32, kind="ExternalInput").ap()
    # weights in natural lhsT layout [c, f], split into two 128-c chunks
    wqk_d = nc.dram_tensor("wqk", [2, 128, 256], F32, kind="ExternalInput").ap()
    wv_d = nc.dram_tensor("wv", [2, 128, 256], F32, kind="ExternalInput").ap()
    bqk_d = bv_d = None
    if with_bias:
        bqk_d = nc.dram_tensor("bqk", [2, 128], F32, kind="ExternalInput").ap()
        bv_d = nc.dram_tensor("bv", [256], F32, kind="ExternalInput").ap()
    out_d = nc.dram_tensor("out", [NS, W, C], F32, kind="ExternalOutput").ap()

    # per-slab views: [p=128, t(w-chunk)=2, c=256]
    x_r = x_d.rearrange("s (t p) c -> s p t c", p=128)
    out_r = out_d.rearrange("s (t p) c -> s p t c", p=128)

    with tile.TileContext(nc) as tc:
        _emit(nc, tc, x_r, out_r, wqk_d, wv_d, bqk_d, bv_d)
    nc.compile()
    return nc


def _emit(nc, tc, x_r, out_r, wqk_d, wv_d, bqk_d, bv_d):
    from contextlib import ExitStack

    with ExitStack() as ctx:
        ec = ctx.enter_context
        consts = ec(tc.tile_pool(name="consts", bufs=1))
        xpool = ec(tc.tile_pool(name="xp", bufs=5))
        xnpool = ec(tc.tile_pool(name="xnp", bufs=3))
        xtpool = ec(tc.tile_pool(name="xtp", bufs=3))
        qkpool = ec(tc.tile_pool(name="qkp", bufs=3))
        epool = ec(tc.tile_pool(name="ep", bufs=3))
        vpool = ec(tc.tile_pool(name="vp", bufs=3))
        opool = ec(tc.tile_pool(name="op", bufs=4))
        stat = ec(tc.tile_pool(name="stat", bufs=6))
        ps_xnT = ec(tc.tile_pool(name="ps_xnT", bufs=2, space="PSUM"))
        ps_qk = ec(tc.tile_pool(name="ps_qk", bufs=1, space="PSUM"))
        ps_sT = ec(tc.tile_pool(name="ps_sT", bufs=1, space="PSUM"))
        ps_v = ec(tc.tile_pool(name="ps_v", bufs=2, space="PSUM"))
        ps_y = ec(tc.tile_pool(name="ps_y", bufs=1, space="PSUM"))

        ident = consts.tile([128, 128], F32)
        make_identity(nc, ident)
        identr = consts.tile([128, 128], F32R)
        nc.vector.tensor_copy(identr, ident)
        negshift = consts.tile([128, 1], F32)
        nc.vector.memset(negshift, -SHIFT)
        eps_t = consts.tile([128, 1], F32)
        nc.vector.memset(eps_t, EPS)
        ones2 = consts.tile([128, 2], F32)
        nc.vector.memset(ones2, 1.0)

        wqk_f = consts.tile([128, 2, 256], F32)
        nc.sync.dma_start(wqk_f, wqk_d.rearrange("t p f -> p t f"))
        wv_f = consts.tile([128, 2, 256], F32)
        nc.sync.dma_start(wv_f, wv_d.rearrange("t p f -> p t f"))
        wqk = consts.tile([128, 2, 256], F32R)
        nc.vector.tensor_copy(wqk, wqk_f)
        wv = consts.tile([128, 2, 256], F32R)
        nc.vector.tensor_copy(wv, wv_f)

        if bqk_d is not None:
            import concourse.bass as bass
            bqk_sb = consts.tile([128, 2], F32)
            nc.sync.dma_start(bqk_sb, bqk_d.rearrange("t p -> p t"))
            bvf = consts.tile([128, 2, 256], F32)
            bv_b = bass.AP(tensor=bv_d.tensor, offset=bv_d.offset,
                           ap=[[0, 128], [0, 2], [1, 256]])
            nc.sync.dma_start(bvf, bv_b)

        def emit_tail(pv):
            """Slab tail, software-pipelined: y-matmul, 1/Z normalize,
            skip-add, store."""
            E, v_sb, x_sb, s = pv["E"], pv["v"], pv["x_sb"], pv["s"]
            p_y = ps_y.tile([128, 2, 258], F32)
            for it in (0, 1):
                for jt in (0, 1):
                    nc.tensor.matmul(
                        p_y[:, it, 0:258],
                        E[:, jt, it * 128:(it + 1) * 128],
                        v_sb[:, jt, 0:258],
                        start=(jt == 0), stop=(jt == 1))
            rZ = stat.tile([128, 2, 1], F32)
            nc.vector.reciprocal(rZ, p_y[:, :, 256:257])
            tmp = opool.tile([128, 2, 256], F32)
            for it in (0, 1):
                nc.scalar.mul(tmp[:, it, :], p_y[:, it, 0:256], rZ[:, it, :])
            o_sb = opool.tile([128, 2, 256], F32)
            nc.gpsimd.tensor_tensor(out=o_sb, in0=tmp, in1=x_sb, op=ALU.add)
            if bv_d is not None:
                nc.gpsimd.tensor_tensor(out=o_sb, in0=o_sb, in1=bvf,
                                        op=ALU.add)
            nc.sync.dma_start(out_r[s], o_sb)

        def emit_scores(pq):
            """Scores stage, software-pipelined one slab behind: s^T matmul,
            exp -> E (f32r, feeds y-matmul directly)."""
            qkT = pq["qkT"]
            p_sT = ps_sT.tile([128, 2, 256], F32)
            for jt in (0, 1):
                nc.tensor.matmul(
                    p_sT[:, jt, :],
                    qkT[:, 1, jt * 128:(jt + 1) * 128],
                    qkT[:, 0, :],
                    start=True, stop=True)
            E = epool.tile([128, 2, 256], F32R)
            nc.scalar.activation(out=E, in_=p_sT, func=AF.Exp,
                                 bias=negshift, scale=1.0)
            return {"E": E, "v": pq["v"], "x_sb": pq["x_sb"], "s": pq["s"]}

        prev = None
        prevq = None
        for s in range(NS):
            x_sb = xpool.tile([128, 2, 256], F32)
            nc.sync.dma_start(x_sb, x_r[s])

            # LayerNorm stats per row (partition = w position)
            st = stat.tile([128, 2, 6], F32)
            mv = stat.tile([128, 2, 2], F32)
            for t in (0, 1):
                nc.vector.bn_stats(st[:, t, :], x_sb[:, t, :])
                nc.vector.bn_aggr(mv[:, t, :], st[:, t, :])
            # rs = rsqrt(var + eps) = exp(-0.5 * ln(var + eps)); ln+exp live
            # in one ACT table set (see _install_act_root)
            lnv = stat.tile([128, 2, 1], F32)
            nc.scalar.activation(out=lnv, in_=mv[:, :, 1:2], func=AF.Ln,
                                 bias=eps_t, scale=1.0)
            rs = stat.tile([128, 2, 1], F32)
            nc.scalar.activation(out=rs, in_=lnv, func=AF.Exp, scale=-0.5)
            # nmr = -mu * rs (per-partition bias for the fused LN apply)
            nmr = stat.tile([128, 2, 1], F32)
            for t in (0, 1):
                nc.vector.tensor_scalar(
                    out=nmr[:, t, :], in0=mv[:, t, 0:1],
                    scalar1=rs[:, t, :], scalar2=-1.0,
                    op0=ALU.mult, op1=ALU.mult)

            # xn = (x - mu) * rs in one fused ACT op per w-chunk
            # (Identity allows AP bias+scale; same table set as Exp/Ln)
            xn = xnpool.tile([128, 2, 256], F32R)
            for t in (0, 1):
                nc.scalar.activation(out=xn[:, t, :], in_=x_sb[:, t, :],
                                     func=AF.Identity, bias=nmr[:, t, :],
                                     scale=rs[:, t, :])

            # transpose xn -> [c, w] layout (PE transpose, f32r 1.5 cyc/row)
            p_xnT = ps_xnT.tile([128, 2, 256], F32R)
            for cc in (0, 1):
                for t in (0, 1):
                    nc.tensor.transpose(
                        p_xnT[:, cc, t * 128:(t + 1) * 128],
                        xn[:, t, cc * 128:(cc + 1) * 128], identr)
            xnT = xtpool.tile([128, 2, 256], F32R)
            nc.vector.tensor_copy(xnT, p_xnT)

            # previous slab's tail fills PE while stats/evicts run here
            if prev is not None:
                emit_tail(prev)

            # qk^T = Wqk^T @ xn^T  (single f32r matmul per block)
            p_qk = ps_qk.tile([128, 2, 256], F32)
            for blk in (0, 1):
                for cc in (0, 1):
                    nc.tensor.matmul(
                        p_qk[:, blk, :],
                        wqk[:, cc, blk * 128:(blk + 1) * 128],
                        xnT[:, cc, :],
                        start=(cc == 0), stop=(cc == 1))
            if bqk_d is not None:
                for blk in (0, 1):
                    nc.vector.tensor_scalar(
                        out=p_qk[:, blk, :], in0=p_qk[:, blk, :],
                        scalar1=bqk_sb[:, blk:blk + 1], scalar2=None,
                        op0=ALU.add)
            qkT = qkpool.tile([128, 2, 256], F32R)
            nc.vector.tensor_copy(qkT, p_qk)

            # v = xn @ Wv  (single f32r matmul per block)
            p_v = ps_v.tile([128, 2, 256], F32)
            for jt in (0, 1):
                for cc in (0, 1):
                    nc.tensor.matmul(
                        p_v[:, jt, :],
                        xnT[:, cc, jt * 128:(jt + 1) * 128],
                        wv[:, cc, :],
                        start=(cc == 0), stop=(cc == 1))
            v_sb = vpool.tile([128, 2, 258], F32R)
            nc.gpsimd.tensor_copy(v_sb[:, :, 0:256], p_v)
            # ones columns accumulate Z in the y-matmul
            nc.vector.tensor_copy(v_sb[:, :, 256:258], ones2)

            if prevq is not None:
                prev = emit_scores(prevq)

            prevq = {"qkT": qkT, "v": v_sb, "x_sb": x_sb, "s": s}
        emit_tail(prev)
        prev = emit_scores(prevq)
        emit_tail(prev)


def _install_ntff_hook():
    """Register the axon NTFF profiling hook (the image's antenv lacks
    axon_hooks, so boot skipped registration). Trace-only; best-effort."""
    try:
        import types

        import antenv

        if getattr(antenv, "axon_hooks", None) is not None:
            return
        mod = types.ModuleType("antenv.axon_hooks")
        _h = [None]
        mod.set_axon_ntff_profile_hook = lambda h: _h.__setitem__(0, h)
        mod.get_axon_ntff_profile_hook = lambda: _h[0]
        sys.modules["antenv.axon_hooks"] = mod
        antenv.axon_hooks = mod
        from trn_agent_boot.trn_boot import _ntff_profile_via_ctypes

        hook = _ntff_profile_via_ctypes("/opt/axon/libaxon_pjrt.so")
        if hook is not None:
            mod.set_axon_ntff_profile_hook(hook)
    except Exception as e:  # noqa: BLE001
        print(f"ntff hook install failed (timing unavailable): {e}")


def kernel(x, ln_gamma, ln_beta, W_qkv):
    x = np.asarray(x, dtype=np.float32)
    ln_gamma = np.asarray(ln_gamma, dtype=np.float32)
    ln_beta = np.asarray(ln_beta, dtype=np.float32)
    W_qkv = np.asarray(W_qkv, dtype=np.float32)
    assert x.shape == (B, H, W, C) and W_qkv.shape == (C, F2)

    # fold gamma/beta into the projection (1x1 conv has no bias of its own)
    Wp = (ln_gamma.astype(np.float64)[:, None] * W_qkv.astype(np.float64))
    bW = (ln_beta.astype(np.float64) @ W_qkv.astype(np.float64)).astype(np.float32)
    with_bias = bool(np.any(bW != 0.0))

    key = with_bias
    if key not in _NC_CACHE:
        _NC_CACHE[key] = _build(with_bias)
    nc = _NC_CACHE[key]

    wqk = np.ascontiguousarray(
        Wp[:, :256].astype(np.float32).reshape(2, 128, 256))
    wv = np.ascontiguousarray(
        Wp[:, 256:].astype(np.float32).reshape(2, 128, 256))
    in_maps = []
    for b in range(B):
        m = {"x": np.ascontiguousarray(x[b]), "wqk": wqk, "wv": wv}
        if with_bias:
            m["bqk"] = np.ascontiguousarray(bW[:256].reshape(2, 128))
            m["bv"] = np.ascontiguousarray(bW[256:])
        in_maps.append(m)

    trace = os.environ.get("KERNEL_TRACE", "") == "1"
    if trace:
        _install_ntff_hook()
    res = run_bass_kernel_spmd(nc, in_maps, core_ids=list(range(B)), trace=trace)
    if trace and res.exec_time_ns is not None:
        print(f"HW exec time: {res.exec_time_ns} ns")
        if res.instructions_and_trace is not None:
            print(f"trace: {res.instructions_and_trace[1]}")
    out = np.stack([res.results[b]["out"] for b in range(B)], axis=0)
    return out.reshape(B, H, W, C).astype(np.float32, copy=False)
